# revision 6
# baseline (speedup 1.0000x reference)
"""Trainium2 Bass kernel for nn_Discriminator (GIN message passing + pool + FC).

Strategy (8 NeuronCores, SPMD):
- Nodes sharded by graph boundaries (graphs 50c..50(c+1) -> core c), so
  global_add_pool and the FC tail are fully core-local.
- Message passing: edges assigned to the core owning dst; edges sorted by dst;
  gather h[src] rows via SWDGE dma_gather (random 1KB rows, bf16); scatter-add
  via one-hot matmuls on the TensorEngine accumulating in PSUM. "+h" (GIN eps=0)
  is realized with self-edges, so u = h + A@h comes out of PSUM directly.
- MLP per layer runs H-major (activations transposed via PE transposes):
  z1 = PReLU(u@w1 + b1) [bias via K=1 matmul; PReLU = max(v, a*v) on DVE],
  h  = relu(z1@w2' + b2') [BatchNorm folded into w2/b2 on host; ACT Relu+bias].
- Between layers, h (bf16) is exchanged with an ncfw AllGather into a Shared
  DRAM buffer which doubles as the next layer's gather table.
- Tail: pooling via one-hot matmul (graph-local), FC1/FC2 with stats/adj
  pre-transposed on host; outputs per-core [50 graphs] slices, host concats.
"""
import sys
sys.path.insert(0, '/opt/trn_rl_repo')

import numpy as np
import ml_dtypes

import concourse.bacc as bacc
import concourse.mybir as mybir
import concourse.tile as tile
from concourse.masks import make_identity

P = 128
NCORES = 8
CHUNK = 128     # edges per scatter matmul
GB = 8          # chunks per dma_gather batch
L = 3
BN_EPS = 1e-5
PGT = 64        # padded graphs per core

BF = ml_dtypes.bfloat16


# ----------------------------------------------------------------- host prep
def preprocess(inputs):
    x = np.asarray(inputs["x"], np.float32)
    ei = np.asarray(inputs["edge_index"], np.int64)
    batch = np.asarray(inputs["batch"], np.int64)
    N, H = x.shape
    E = ei.shape[1]
    G = int(np.asarray(inputs["stats"]).shape[0])
    KT = H // P
    gper = G // NCORES
    assert G % NCORES == 0 and H % P == 0

    # --- graph-aligned node ranges
    n0 = np.searchsorted(batch, np.arange(NCORES) * gper, side="left")
    n1 = np.append(n0[1:], N)
    cnt = n1 - n0
    NT = int(np.ceil(cnt.max() / P))
    NLOCP = NT * P
    assert NCORES * NLOCP < 32768, "int16 gather index overflow"

    # padded-row mapping: global node -> padded row
    owner = np.searchsorted(n0, np.arange(N), side="right") - 1
    prow = owner * NLOCP + (np.arange(N) - n0[owner])

    x_pad = np.zeros((NCORES * NLOCP, H), np.float32)
    x_pad[prow] = x
    x_pad_bf = x_pad.astype(BF)

    # --- edges per core (dst-owner), with self-edges, sorted by dst
    src, dst = ei[0], ei[1]
    e_owner = owner[dst]
    # self edges
    all_src = np.concatenate([src, np.arange(N)])
    all_dst = np.concatenate([dst, np.arange(N)])
    all_owner = np.concatenate([e_owner, owner])

    nchunk_t = np.zeros(NT, np.int64)
    per_core = []
    for c in range(NCORES):
        m = all_owner == c
        s_c = prow[all_src[m]]
        d_c = prow[all_dst[m]] - c * NLOCP
        o = np.argsort(d_c, kind="stable")
        s_c, d_c = s_c[o], d_c[o]
        t_c = d_c // P
        counts = np.bincount(t_c, minlength=NT)
        nchunk_t = np.maximum(nchunk_t, (counts + CHUNK - 1) // CHUNK)
        per_core.append((s_c, d_c, counts))
    nchunk_t = np.maximum(nchunk_t, 1)
    NCHUNKS = int(nchunk_t.sum())
    EPC = NCHUNKS * CHUNK

    gidx = np.zeros((NCORES, EPC), np.int16)
    dstl = np.full((NCORES, EPC), -1.0, np.float32)
    qt0 = np.concatenate([[0], np.cumsum(nchunk_t)])[:-1]  # first chunk of tile t
    for c in range(NCORES):
        s_c, d_c, counts = per_core[c]
        off = np.concatenate([[0], np.cumsum(counts)])
        for t in range(NT):
            seg = slice(off[t], off[t + 1])
            k = counts[t]
            base = qt0[t] * CHUNK
            gidx[c, base:base + k] = s_c[seg]
            dstl[c, base:base + k] = (d_c[seg] - t * P).astype(np.float32)

    # wrap indices: [128, EPC//16] int16 (16-partition wrap, replicated 8x)
    def wrap16(a):
        w = a.reshape(-1, 16).T  # [16, EPC/16]
        return np.tile(w, (8, 1)).copy()

    gidx_w = np.stack([wrap16(gidx[c]) for c in range(NCORES)])
    # dstl per chunk column: [128, NCHUNKS]
    dstl_t = dstl.reshape(NCORES, NCHUNKS, CHUNK).transpose(0, 2, 1).astype(np.float32).copy()

    # pool graph-locals per node row: [128, NT]
    pgl = np.full((NCORES, NLOCP), -1.0, np.float32)
    for c in range(NCORES):
        g_loc = batch[n0[c]:n1[c]] - gper * c
        assert g_loc.min() >= 0 and g_loc.max() < PGT
        pgl[c, :cnt[c]] = g_loc
    pgl_t = pgl.reshape(NCORES, NT, P).transpose(0, 2, 1).astype(np.float32).copy()

    # --- weights
    w1 = np.asarray(inputs["w1"], np.float64)
    b1 = np.asarray(inputs["b1"], np.float64)
    a1 = np.asarray(inputs["a1"], np.float64)
    gam = np.asarray(inputs["bn_gamma"], np.float64)
    bet = np.asarray(inputs["bn_beta"], np.float64)
    mu = np.asarray(inputs["bn_mean"], np.float64)
    var = np.asarray(inputs["bn_var"], np.float64)
    w2 = np.asarray(inputs["w2"], np.float64)
    b2 = np.asarray(inputs["b2"], np.float64)
    a2 = np.asarray(inputs["a2"], np.float64)
    assert np.all(a2 >= 0), "relu(prelu) fusion needs a2 >= 0"

    s = gam / np.sqrt(var + BN_EPS)          # [L, H]
    tt = bet - mu * s
    w2f = s[:, :, None] * w2                 # fold BN into w2
    b2f = np.einsum("lh,lho->lo", tt, w2) + b2

    w1t = w1.reshape(L, KT, P, H).astype(BF)
    w2t = w2f.reshape(L, KT, P, H).astype(BF)
    b1t = b1.reshape(L, 1, H).astype(BF)
    b2t = np.ascontiguousarray(
        b2f.reshape(L, KT, P).transpose(2, 0, 1).reshape(P, L * KT)
    ).astype(np.float32)

    # --- fc tail
    fc1_w = np.asarray(inputs["fc1_w"], np.float32)   # [H + 8 + NMAX^2, 256]
    fc1_b = np.asarray(inputs["fc1_b"], np.float32)
    fc2_w = np.asarray(inputs["fc2_w"], np.float32)   # [256, 1]
    fc2_b = float(np.asarray(inputs["fc2_b"]).reshape(-1)[0])
    stats = np.asarray(inputs["stats"], np.float32)
    adj = np.asarray(inputs["adj"], np.float32)
    NC2 = stats.shape[1]
    AD = adj.shape[1] * adj.shape[2]
    FO = fc1_w.shape[1]
    ADP = ((AD + P - 1) // P) * P
    KADJ = ADP // P
    KF = KT + 1 + KADJ
    fc1wp = np.zeros((KF * P, FO), np.float32)
    fc1wp[:H] = fc1_w[:H]
    fc1wp[H:H + NC2] = fc1_w[H:H + NC2]
    fc1wp[H + P:H + P + AD] = fc1_w[H + NC2:]
    fc1wp = fc1wp.reshape(KF, P, FO)
    MT = FO // P
    fc1bt = np.ascontiguousarray(fc1_b.reshape(MT, P).T).astype(np.float32)
    fc2wp = np.ascontiguousarray(fc2_w.reshape(2, P).T).astype(np.float32)

    statst = np.zeros((NCORES, P, PGT), np.float32)
    adjt = np.zeros((NCORES, KADJ, P, PGT), np.float32)
    for c in range(NCORES):
        statst[c, :NC2, :gper] = stats[gper * c:gper * (c + 1)].T
        a = adj[gper * c:gper * (c + 1)].reshape(gper, AD).T  # [AD, gper]
        adjt[c, :, :, :gper] = np.pad(a, ((0, ADP - AD), (0, 0))).reshape(KADJ, P, gper)

    meta = dict(
        N=N, H=H, KT=KT, G=G, gper=gper, NT=NT, NLOCP=NLOCP,
        NCHUNKS=NCHUNKS, EPC=EPC, nchunk_t=nchunk_t.tolist(),
        qt0=qt0.tolist(), a1=[float(v) for v in a1], fc2_b=fc2_b,
        KF=KF, KADJ=KADJ, MT=MT, FO=FO, cnt=cnt.tolist(),
    )
    shared = dict(
        xin=x_pad_bf, w1t=w1t, w2t=w2t, b1t=b1t, b2t=b2t,
        fc1wp=fc1wp, fc1bt=fc1bt, fc2wp=fc2wp,
        fc2bt=np.full((1, 1), fc2_b, np.float32),
    )
    in_maps = []
    for c in range(NCORES):
        m = dict(shared)
        m.update(gidx=gidx_w[c], dstl=dstl_t[c], pgl=pgl_t[c],
                 statst=statst[c], adjt=adjt[c])
        in_maps.append(m)
    return meta, in_maps


# ----------------------------------------------------------------- device build
def build_nc(meta):
    H, KT, NT, NLOCP = meta["H"], meta["KT"], meta["NT"], meta["NLOCP"]
    NCHUNKS, EPC = meta["NCHUNKS"], meta["EPC"]
    nchunk_t, qt0 = meta["nchunk_t"], meta["qt0"]
    KF, KADJ, MT, FO = meta["KF"], meta["KADJ"], meta["MT"], meta["FO"]
    f32, bf16, i16 = mybir.dt.float32, mybir.dt.bfloat16, mybir.dt.int16
    AL = mybir.AluOpType

    nc = bacc.Bacc()
    xin = nc.dram_tensor("xin", [NCORES * NLOCP, H], bf16, kind="ExternalInput")
    gidx = nc.dram_tensor("gidx", [P, EPC // 16], i16, kind="ExternalInput")
    dstl = nc.dram_tensor("dstl", [P, NCHUNKS], f32, kind="ExternalInput")
    pgl = nc.dram_tensor("pgl", [P, NT], f32, kind="ExternalInput")
    w1t = nc.dram_tensor("w1t", [L, KT, P, H], bf16, kind="ExternalInput")
    w2t = nc.dram_tensor("w2t", [L, KT, P, H], bf16, kind="ExternalInput")
    b1t = nc.dram_tensor("b1t", [L, 1, H], bf16, kind="ExternalInput")
    b2t = nc.dram_tensor("b2t", [P, L * KT], f32, kind="ExternalInput")
    fc1wp = nc.dram_tensor("fc1wp", [KF, P, FO], f32, kind="ExternalInput")
    fc1bt = nc.dram_tensor("fc1bt", [P, MT], f32, kind="ExternalInput")
    fc2wp = nc.dram_tensor("fc2wp", [P, 2], f32, kind="ExternalInput")
    fc2bt = nc.dram_tensor("fc2bt", [1, 1], f32, kind="ExternalInput")
    statst = nc.dram_tensor("statst", [P, PGT], f32, kind="ExternalInput")
    adjt = nc.dram_tensor("adjt", [KADJ, P, PGT], f32, kind="ExternalInput")

    hb = nc.dram_tensor("hb", [NLOCP, H], bf16)
    gath = [None,
            nc.dram_tensor("gath1", [NCORES * NLOCP, H], bf16, addr_space="Shared"),
            nc.dram_tensor("gath2", [NCORES * NLOCP, H], bf16, addr_space="Shared")]

    out_g = nc.dram_tensor("out_g", [1, PGT], f32, kind="ExternalOutput")
    xl_g = nc.dram_tensor("xl_g", [PGT, FO], f32, kind="ExternalOutput")

    cc_sem = nc.alloc_semaphore("cc_sem")

    with tile.TileContext(nc) as tc:
        with (
            tc.tile_pool(name="const", bufs=1) as cp,
            tc.tile_pool(name="wpool", bufs=1) as wp,
            tc.tile_pool(name="msgs", bufs=3) as mp,
            tc.tile_pool(name="oh", bufs=4) as ohp,
            tc.tile_pool(name="work", bufs=2) as wk,
            tc.tile_pool(name="chunk", bufs=2) as ck,
            tc.tile_pool(name="psu", bufs=2, space="PSUM") as pp_u,
            tc.tile_pool(name="pst", bufs=1, space="PSUM") as pp_t,
            tc.tile_pool(name="psz", bufs=2, space="PSUM") as pp_z,
            tc.tile_pool(name="psh", bufs=1, space="PSUM") as pp_h,
            tc.tile_pool(name="psp", bufs=1, space="PSUM") as pp_p,
        ):
            # ---- constants / resident tensors
            gidx_sb = cp.tile([P, EPC // 16], i16)
            nc.sync.dma_start(gidx_sb[:], gidx[:])
            dstl_sb = cp.tile([P, NCHUNKS], f32)
            nc.sync.dma_start(dstl_sb[:], dstl[:])
            pgl_sb = cp.tile([P, NT], f32)
            nc.sync.dma_start(pgl_sb[:], pgl[:])
            iota_i = cp.tile([P, P], mybir.dt.int32)
            nc.gpsimd.iota(iota_i[:], pattern=[[1, P]], base=0, channel_multiplier=0)
            iota_bf = cp.tile([P, P], bf16)
            nc.vector.tensor_copy(iota_bf[:], iota_i[:])
            ident_bf = cp.tile([P, P], bf16)
            make_identity(nc, ident_bf[:])
            ident_f = cp.tile([P, P], f32)
            make_identity(nc, ident_f[:])
            ones_sb = cp.tile([1, 512], bf16)
            nc.gpsimd.memset(ones_sb[:], 1.0)

            w1_sb = wp.tile([P, L * KT * H], bf16)
            w2_sb = wp.tile([P, L * KT * H], bf16)
            for l_ in range(L):
                for kt in range(KT):
                    o = (l_ * KT + kt) * H
                    nc.sync.dma_start(w1_sb[:, o:o + H], w1t[l_, kt, :, :])
                    nc.sync.dma_start(w2_sb[:, o:o + H], w2t[l_, kt, :, :])
            b1_sb = wp.tile([1, L * H], bf16)
            for l_ in range(L):
                nc.sync.dma_start(b1_sb[0:1, l_ * H:(l_ + 1) * H], b1t[l_, :, :])
            b2_sb = wp.tile([P, L * KT], f32)
            nc.sync.dma_start(b2_sb[:], b2t[:])

            psum_pool = pp_p.tile([PGT, H], f32, space="PSUM")

            # ---- layers
            for l in range(L):
                src_d = xin if l == 0 else gath[l]
                a1l = meta["a1"][l]
                nbatch = (NCHUNKS + GB - 1) // GB
                msg_tiles = {}

                def get_msgs(b):
                    if b not in msg_tiles:
                        nb = min(GB, NCHUNKS - b * GB)
                        mt_ = mp.tile([P, GB, H], bf16, tag="msgs")
                        nc.gpsimd.dma_gather(
                            mt_[:, :nb, :], src_d[:],
                            gidx_sb[:, b * GB * CHUNK // 16:
                                    (b * GB + nb) * CHUNK // 16],
                            nb * CHUNK, nb * CHUNK, H)
                        msg_tiles[b] = mt_
                        if len(msg_tiles) > 3:
                            del msg_tiles[min(msg_tiles)]
                    return msg_tiles[b]

                nchk = (NT + 3) // 4
                for blk in range(nchk):
                    t_lo = blk * 4
                    t_hi = min(t_lo + 4, NT)
                    ntile = t_hi - t_lo
                    fd = ntile * P
                    uTc = ck.tile([P, KT, 512], bf16, tag="uTc")
                    for t in range(t_lo, t_hi):
                        psum_u = pp_u.tile([P, H], f32, space="PSUM", tag="u")
                        for ci in range(nchunk_t[t]):
                            q = qt0[t] + ci
                            mt_ = get_msgs(q // GB)
                            oh = ohp.tile([P, P], bf16, tag="oh")
                            nc.vector.tensor_scalar(
                                out=oh[:], in0=iota_bf[:],
                                scalar1=dstl_sb[:, q:q + 1], scalar2=None,
                                op0=AL.is_equal)
                            nc.tensor.matmul(
                                psum_u[:], lhsT=oh[:], rhs=mt_[:, q % GB, :],
                                start=(ci == 0), stop=(ci == nchunk_t[t] - 1))
                        u_sb = wk.tile([P, H], bf16, tag="u_sb")
                        nc.scalar.activation(u_sb[:], psum_u[:],
                                             mybir.ActivationFunctionType.Copy)
                        psum_uT = pp_t.tile([P, H], bf16, space="PSUM", tag="uT")
                        for kt in range(KT):
                            nc.tensor.transpose(
                                psum_uT[:, kt * P:(kt + 1) * P],
                                u_sb[:, kt * P:(kt + 1) * P], ident_bf[:])
                        for kt in range(KT):
                            nc.scalar.activation(
                                uTc[:, kt, (t - t_lo) * P:(t - t_lo + 1) * P],
                                psum_uT[:, kt * P:(kt + 1) * P],
                                mybir.ActivationFunctionType.Copy)
                    # MLP on this node block (H-major, free dim = fd)
                    z1T = ck.tile([P, KT, 512], bf16, tag="z1T")
                    for mt_i in range(KT):
                        psum_z = pp_z.tile([P, 512], f32, space="PSUM", tag="z")
                        for kt in range(KT):
                            nc.tensor.matmul(
                                psum_z[:, :fd],
                                lhsT=w1_sb[:, (l * KT + kt) * H + mt_i * P:
                                           (l * KT + kt) * H + (mt_i + 1) * P],
                                rhs=uTc[:, kt, :fd], start=(kt == 0), stop=False)
                        nc.tensor.matmul(
                            psum_z[:, :fd],
                            lhsT=b1_sb[0:1, l * H + mt_i * P:l * H + (mt_i + 1) * P],
                            rhs=ones_sb[0:1, :fd], start=False, stop=True)
                        t2 = wk.tile([P, 512], bf16, tag="t2")
                        nc.vector.tensor_scalar(
                            out=t2[:, :fd], in0=psum_z[:, :fd], scalar1=a1l,
                            scalar2=None, op0=AL.mult)
                        nc.vector.tensor_tensor(
                            out=z1T[:, mt_i, :fd], in0=psum_z[:, :fd],
                            in1=t2[:, :fd], op=AL.max)
                    hT = ck.tile([P, KT, 512], bf16, tag="hT")
                    for mt_i in range(KT):
                        psum_z = pp_z.tile([P, 512], f32, space="PSUM", tag="z")
                        for kt in range(KT):
                            nc.tensor.matmul(
                                psum_z[:, :fd],
                                lhsT=w2_sb[:, (l * KT + kt) * H + mt_i * P:
                                           (l * KT + kt) * H + (mt_i + 1) * P],
                                rhs=z1T[:, kt, :fd],
                                start=(kt == 0), stop=(kt == KT - 1))
                        nc.scalar.activation(
                            hT[:, mt_i, :fd], psum_z[:, :fd],
                            mybir.ActivationFunctionType.Relu,
                            bias=b2_sb[:, l * KT + mt_i:l * KT + mt_i + 1])
                    # back-transpose to node-major; ship or pool
                    for ti in range(ntile):
                        t0 = t_lo + ti
                        psum_h = pp_h.tile([P, H], bf16, space="PSUM", tag="hn")
                        for kt in range(KT):
                            nc.tensor.transpose(
                                psum_h[:, kt * P:(kt + 1) * P],
                                hT[:, kt, ti * P:(ti + 1) * P], ident_bf[:])
                        h_sb = wk.tile([P, H], bf16, tag="h_sb")
                        nc.scalar.activation(h_sb[:], psum_h[:],
                                             mybir.ActivationFunctionType.Copy)
                        if l < L - 1:
                            nc.sync.dma_start(hb[t0 * P:(t0 + 1) * P, :], h_sb[:])
                        else:
                            poh = ohp.tile([P, PGT], bf16, tag="poh")
                            nc.vector.tensor_scalar(
                                out=poh[:], in0=iota_bf[:, :PGT],
                                scalar1=pgl_sb[:, t0:t0 + 1], scalar2=None,
                                op0=AL.is_equal)
                            nc.tensor.matmul(
                                psum_pool[:], lhsT=poh[:], rhs=h_sb[:],
                                start=(t0 == 0), stop=(t0 == NT - 1))
                if l < L - 1:
                    with tc.tile_critical():
                        nc.gpsimd.collective_compute(
                            "AllGather", AL.bypass,
                            replica_groups=[list(range(NCORES))],
                            ins=[hb[:]], outs=[gath[l + 1][:]],
                        ).then_inc(cc_sem, 1)
                        nc.gpsimd.wait_ge(cc_sem, l + 1)

            # ---- tail: pooled -> fc1 -> fc2
            pooled_sb = wk.tile([PGT, H], f32, tag="pooled")
            nc.scalar.activation(pooled_sb[:], psum_pool[:],
                                 mybir.ActivationFunctionType.Copy)
            pooledT = wk.tile([P, KT * PGT], f32, tag="pooledT")
            for kt in range(KT):
                psum_pt = pp_t.tile([P, PGT], f32, space="PSUM", tag="uT")
                nc.tensor.transpose(
                    psum_pt[:], pooled_sb[:, kt * P:(kt + 1) * P],
                    ident_f[:PGT, :PGT])
                nc.scalar.activation(pooledT[:, kt * PGT:(kt + 1) * PGT],
                                     psum_pt[:],
                                     mybir.ActivationFunctionType.Copy)
            statst_sb = wk.tile([P, PGT], f32, tag="statst")
            nc.sync.dma_start(statst_sb[:], statst[:])

            psum_x1 = []
            for _mt in range(MT):
                px = pp_z.tile([P, PGT], f32, space="PSUM", tag="z")
                psum_x1.append(px)
            for kf in range(KF):
                fw = mp.tile([P, FO], f32, tag="fcw")
                nc.sync.dma_start(fw[:], fc1wp[kf, :, :])
                if kf < KT:
                    rhs = pooledT[:, kf * PGT:(kf + 1) * PGT]
                elif kf == KT:
                    rhs = statst_sb[:]
                else:
                    at = mp.tile([P, PGT], f32, tag="adjt")
                    nc.sync.dma_start(at[:], adjt[kf - KT - 1, :, :])
                    rhs = at[:]
                for mt_i in range(MT):
                    nc.tensor.matmul(
                        psum_x1[mt_i][:], lhsT=fw[:, mt_i * P:(mt_i + 1) * P],
                        rhs=rhs, start=(kf == 0), stop=(kf == KF - 1))
            fc1b_sb = wk.tile([P, MT], f32, tag="fc1b")
            nc.sync.dma_start(fc1b_sb[:], fc1bt[:])
            xlT = wk.tile([P, MT * PGT], f32, tag="xlT")
            for mt_i in range(MT):
                nc.scalar.activation(
                    xlT[:, mt_i * PGT:(mt_i + 1) * PGT], psum_x1[mt_i][:],
                    mybir.ActivationFunctionType.Relu,
                    bias=fc1b_sb[:, mt_i:mt_i + 1])
            # fc2 + sigmoid
            fc2w_sb = wk.tile([P, 2], f32, tag="fc2w")
            nc.sync.dma_start(fc2w_sb[:], fc2wp[:])
            psum_o = pp_h.tile([1, PGT], f32, space="PSUM", tag="hn")
            for kt2 in range(MT):
                nc.tensor.matmul(
                    psum_o[:], lhsT=fc2w_sb[:, kt2:kt2 + 1],
                    rhs=xlT[:, kt2 * PGT:(kt2 + 1) * PGT],
                    start=(kt2 == 0), stop=(kt2 == MT - 1))
            fc2b_sb = wk.tile([1, 1], f32, tag="fc2b")
            nc.sync.dma_start(fc2b_sb[:], fc2bt[:])
            o_sb = wk.tile([1, PGT], f32, tag="o_sb")
            nc.scalar.activation(o_sb[:], psum_o[:],
                                 mybir.ActivationFunctionType.Sigmoid,
                                 bias=fc2b_sb[:])
            nc.sync.dma_start(out_g[:], o_sb[:])
            # x_l: transpose back to graph-major
            for mt_i in range(MT):
                psum_xl = pp_u.tile([PGT, P], f32, space="PSUM", tag="u")
                nc.tensor.transpose(
                    psum_xl[:], xlT[:, mt_i * PGT:(mt_i + 1) * PGT], ident_f[:])
                xl_sb = wk.tile([PGT, P], f32, tag="xl_sb")
                nc.scalar.activation(xl_sb[:], psum_xl[:],
                                     mybir.ActivationFunctionType.Copy)
                nc.sync.dma_start(xl_g[:, mt_i * P:(mt_i + 1) * P], xl_sb[:])

    nc.has_collectives = True
    nc.compile()
    return nc


# ----------------------------------------------------------------- runner
def _make_runner(nc, n_cores=NCORES):
    import jax
    from jax.sharding import Mesh, PartitionSpec
    from jax.experimental.shard_map import shard_map
    from concourse.bass2jax import (_bass_exec_p, install_neuronx_cc_hook,
                                    partition_id_tensor)
    install_neuronx_cc_hook()
    partition_name = nc.partition_id_tensor.name if nc.partition_id_tensor else None
    in_names, out_names, out_avals, zero_outs = [], [], [], []
    for alloc in nc.m.functions[0].allocations:
        if not isinstance(alloc, mybir.MemoryLocationSet):
            continue
        name = alloc.memorylocations[0].name
        if alloc.kind == "ExternalInput":
            if name != partition_name:
                in_names.append(name)
        elif alloc.kind == "ExternalOutput":
            shape = tuple(alloc.tensor_shape)
            dtype = mybir.dt.np(alloc.dtype)
            out_names.append(name)
            out_avals.append(jax.core.ShapedArray(shape, dtype))
            zero_outs.append(np.zeros(shape, dtype))
    n_params = len(in_names)
    n_outs = len(out_avals)
    all_in = list(in_names) + list(out_names)
    if partition_name is not None:
        all_in.append(partition_name)

    def _body(*args):
        operands = list(args)
        if partition_name is not None:
            operands.append(partition_id_tensor())
        return tuple(_bass_exec_p.bind(
            *operands, out_avals=tuple(out_avals), in_names=tuple(all_in),
            out_names=tuple(out_names), lowering_input_output_aliases=(),
            sim_require_finite=False, sim_require_nnan=False, nc=nc))

    devices = jax.devices()[:n_cores]
    mesh = Mesh(np.asarray(devices), ("core",))
    sharded = jax.jit(
        shard_map(_body, mesh=mesh,
                  in_specs=(PartitionSpec("core"),) * (n_params + n_outs),
                  out_specs=(PartitionSpec("core"),) * n_outs,
                  check_rep=False),
        donate_argnums=tuple(range(n_params, n_params + n_outs)),
        keep_unused=True)

    def run(in_maps):
        import jax as _jax
        per_core = [[np.asarray(m[k]) for k in in_names] for m in in_maps]
        concat_in = [np.concatenate([per_core[c][i] for c in range(n_cores)], axis=0)
                     for i in range(n_params)]
        zeros = [np.zeros((n_cores * z.shape[0], *z.shape[1:]), z.dtype)
                 for z in zero_outs]
        outs = sharded(*concat_in, *zeros)
        _jax.block_until_ready(outs)
        return [
            {k: np.asarray(outs[i]).reshape(n_cores, *out_avals[i].shape)[c]
             for i, k in enumerate(out_names)}
            for c in range(n_cores)
        ]
    run.in_names = in_names
    run.out_names = out_names
    run.zero_outs = zero_outs
    run.sharded = sharded
    return run


# ----------------------------------------------------------------- entry point
def kernel(**inputs):
    meta, in_maps = preprocess(inputs)
    nc = build_nc(meta)
    run = _make_runner(nc)
    res = run(in_maps)
    gper = meta["gper"]
    out = np.concatenate([res[c]["out_g"][0, :gper] for c in range(NCORES)])
    out = out.reshape(-1, 1).astype(np.float32)
    x_l = np.concatenate([res[c]["xl_g"][:gper, :] for c in range(NCORES)], axis=0)
    return out, x_l.astype(np.float32)


# revision 7
# speedup vs baseline: 1.1128x; 1.1128x over previous
"""Trainium2 Bass kernel for nn_Discriminator (GIN message passing + pool + FC).

Strategy (8 NeuronCores, SPMD):
- Nodes sharded by graph boundaries (graphs 50c..50(c+1) -> core c), so
  global_add_pool and the FC tail are fully core-local.
- Message passing: edges assigned to the core owning dst; edges sorted by dst;
  gather h[src] rows via SWDGE dma_gather (random 1KB rows, bf16); scatter-add
  via one-hot matmuls on the TensorEngine accumulating in PSUM. "+h" (GIN eps=0)
  is realized with self-edges, so u = h + A@h comes out of PSUM directly.
- MLP per layer runs H-major (activations transposed via PE transposes):
  z1 = PReLU(u@w1 + b1) [bias via K=1 matmul; PReLU = max(v, a*v) on DVE],
  h  = relu(z1@w2' + b2') [BatchNorm folded into w2/b2 on host; ACT Relu+bias].
- Between layers, h (bf16) is exchanged with an ncfw AllGather into a Shared
  DRAM buffer which doubles as the next layer's gather table.
- Tail: pooling via one-hot matmul (graph-local), FC1/FC2 with stats/adj
  pre-transposed on host; outputs per-core [50 graphs] slices, host concats.
"""
import sys
sys.path.insert(0, '/opt/trn_rl_repo')

import numpy as np
import ml_dtypes

import concourse.bacc as bacc
import concourse.mybir as mybir
import concourse.tile as tile
from concourse.masks import make_identity

P = 128
NCORES = 8
CHUNK = 128     # edges per scatter matmul
GB = 8          # chunks per dma_gather batch
L = 3
BN_EPS = 1e-5
PGT = 64        # padded graphs per core

BF = ml_dtypes.bfloat16


# ----------------------------------------------------------------- host prep
def preprocess(inputs):
    x = np.asarray(inputs["x"], np.float32)
    ei = np.asarray(inputs["edge_index"], np.int64)
    batch = np.asarray(inputs["batch"], np.int64)
    N, H = x.shape
    E = ei.shape[1]
    G = int(np.asarray(inputs["stats"]).shape[0])
    KT = H // P
    gper = G // NCORES
    assert G % NCORES == 0 and H % P == 0

    # --- graph-aligned node ranges
    n0 = np.searchsorted(batch, np.arange(NCORES) * gper, side="left")
    n1 = np.append(n0[1:], N)
    cnt = n1 - n0
    NT = int(np.ceil(cnt.max() / P))
    NLOCP = NT * P
    assert NCORES * NLOCP < 32768, "int16 gather index overflow"

    # padded-row mapping: global node -> padded row
    owner = np.searchsorted(n0, np.arange(N), side="right") - 1
    prow = owner * NLOCP + (np.arange(N) - n0[owner])

    x_pad = np.zeros((NCORES * NLOCP, H), np.float32)
    x_pad[prow] = x
    x_pad_bf = x_pad.astype(BF)

    # --- edges per core (dst-owner), with self-edges, sorted by dst
    src, dst = ei[0], ei[1]
    e_owner = owner[dst]
    # self edges
    all_src = np.concatenate([src, np.arange(N)])
    all_dst = np.concatenate([dst, np.arange(N)])
    all_owner = np.concatenate([e_owner, owner])

    nchunk_t = np.zeros(NT, np.int64)
    per_core = []
    for c in range(NCORES):
        m = all_owner == c
        s_c = prow[all_src[m]]
        d_c = prow[all_dst[m]] - c * NLOCP
        o = np.argsort(d_c, kind="stable")
        s_c, d_c = s_c[o], d_c[o]
        t_c = d_c // P
        counts = np.bincount(t_c, minlength=NT)
        nchunk_t = np.maximum(nchunk_t, (counts + CHUNK - 1) // CHUNK)
        per_core.append((s_c, d_c, counts))
    nchunk_t = np.maximum(nchunk_t, 1)
    NCHUNKS = int(nchunk_t.sum())
    EPC = NCHUNKS * CHUNK

    gidx = np.zeros((NCORES, EPC), np.int16)
    dstl = np.full((NCORES, EPC), -1.0, np.float32)
    qt0 = np.concatenate([[0], np.cumsum(nchunk_t)])[:-1]  # first chunk of tile t
    for c in range(NCORES):
        s_c, d_c, counts = per_core[c]
        off = np.concatenate([[0], np.cumsum(counts)])
        for t in range(NT):
            seg = slice(off[t], off[t + 1])
            k = counts[t]
            base = qt0[t] * CHUNK
            gidx[c, base:base + k] = s_c[seg]
            dstl[c, base:base + k] = (d_c[seg] - t * P).astype(np.float32)

    # wrap indices: [128, EPC//16] int16 (16-partition wrap, replicated 8x)
    def wrap16(a):
        w = a.reshape(-1, 16).T  # [16, EPC/16]
        return np.tile(w, (8, 1)).copy()

    gidx_w = np.stack([wrap16(gidx[c]) for c in range(NCORES)])
    # dstl per chunk column: [128, NCHUNKS]
    dstl_t = dstl.reshape(NCORES, NCHUNKS, CHUNK).transpose(0, 2, 1).astype(np.float32).copy()

    # pool graph-locals per node row: [128, NT]
    pgl = np.full((NCORES, NLOCP), -1.0, np.float32)
    for c in range(NCORES):
        g_loc = batch[n0[c]:n1[c]] - gper * c
        assert g_loc.min() >= 0 and g_loc.max() < PGT
        pgl[c, :cnt[c]] = g_loc
    pgl_t = pgl.reshape(NCORES, NT, P).transpose(0, 2, 1).astype(np.float32).copy()

    # --- weights
    w1 = np.asarray(inputs["w1"], np.float64)
    b1 = np.asarray(inputs["b1"], np.float64)
    a1 = np.asarray(inputs["a1"], np.float64)
    gam = np.asarray(inputs["bn_gamma"], np.float64)
    bet = np.asarray(inputs["bn_beta"], np.float64)
    mu = np.asarray(inputs["bn_mean"], np.float64)
    var = np.asarray(inputs["bn_var"], np.float64)
    w2 = np.asarray(inputs["w2"], np.float64)
    b2 = np.asarray(inputs["b2"], np.float64)
    a2 = np.asarray(inputs["a2"], np.float64)
    assert np.all(a2 >= 0), "relu(prelu) fusion needs a2 >= 0"

    s = gam / np.sqrt(var + BN_EPS)          # [L, H]
    tt = bet - mu * s
    w2f = s[:, :, None] * w2                 # fold BN into w2
    b2f = np.einsum("lh,lho->lo", tt, w2) + b2

    w1t = w1.reshape(L, KT, P, H).astype(BF)
    w2t = w2f.reshape(L, KT, P, H).astype(BF)
    b1t = b1.reshape(L, 1, H).astype(BF)
    b2t = np.ascontiguousarray(
        b2f.reshape(L, KT, P).transpose(2, 0, 1).reshape(P, L * KT)
    ).astype(np.float32)

    # --- fc tail
    fc1_w = np.asarray(inputs["fc1_w"], np.float32)   # [H + 8 + NMAX^2, 256]
    fc1_b = np.asarray(inputs["fc1_b"], np.float32)
    fc2_w = np.asarray(inputs["fc2_w"], np.float32)   # [256, 1]
    fc2_b = float(np.asarray(inputs["fc2_b"]).reshape(-1)[0])
    stats = np.asarray(inputs["stats"], np.float32)
    adj = np.asarray(inputs["adj"], np.float32)
    NC2 = stats.shape[1]
    AD = adj.shape[1] * adj.shape[2]
    FO = fc1_w.shape[1]
    ADP = ((AD + P - 1) // P) * P
    KADJ = ADP // P
    KF = KT + 1 + KADJ
    fc1wp = np.zeros((KF * P, FO), np.float32)
    fc1wp[:H] = fc1_w[:H]
    fc1wp[H:H + NC2] = fc1_w[H:H + NC2]
    fc1wp[H + P:H + P + AD] = fc1_w[H + NC2:]
    fc1wp = fc1wp.reshape(KF, P, FO)
    MT = FO // P
    fc1bt = np.ascontiguousarray(fc1_b.reshape(MT, P).T).astype(np.float32)
    fc2wp = np.ascontiguousarray(fc2_w.reshape(2, P).T).astype(np.float32)

    statst = np.zeros((NCORES, P, PGT), np.float32)
    adjt = np.zeros((NCORES, KADJ, P, PGT), np.float32)
    for c in range(NCORES):
        statst[c, :NC2, :gper] = stats[gper * c:gper * (c + 1)].T
        a = adj[gper * c:gper * (c + 1)].reshape(gper, AD).T  # [AD, gper]
        adjt[c, :, :, :gper] = np.pad(a, ((0, ADP - AD), (0, 0))).reshape(KADJ, P, gper)

    meta = dict(
        N=N, H=H, KT=KT, G=G, gper=gper, NT=NT, NLOCP=NLOCP,
        NCHUNKS=NCHUNKS, EPC=EPC, nchunk_t=nchunk_t.tolist(),
        qt0=qt0.tolist(), a1=[float(v) for v in a1], fc2_b=fc2_b,
        KF=KF, KADJ=KADJ, MT=MT, FO=FO, cnt=cnt.tolist(),
    )
    shared = dict(
        xin=x_pad_bf, w1t=w1t, w2t=w2t, b1t=b1t, b2t=b2t,
        fc1wp=fc1wp, fc1bt=fc1bt, fc2wp=fc2wp,
        fc2bt=np.full((1, 1), fc2_b, np.float32),
    )
    in_maps = []
    for c in range(NCORES):
        m = dict(shared)
        m.update(gidx=gidx_w[c], dstl=dstl_t[c], pgl=pgl_t[c],
                 statst=statst[c], adjt=adjt[c])
        in_maps.append(m)
    return meta, in_maps


# ----------------------------------------------------------------- device build
def build_nc(meta):
    H, KT, NT, NLOCP = meta["H"], meta["KT"], meta["NT"], meta["NLOCP"]
    NCHUNKS, EPC = meta["NCHUNKS"], meta["EPC"]
    nchunk_t, qt0 = meta["nchunk_t"], meta["qt0"]
    KF, KADJ, MT, FO = meta["KF"], meta["KADJ"], meta["MT"], meta["FO"]
    f32, bf16, i16 = mybir.dt.float32, mybir.dt.bfloat16, mybir.dt.int16
    AL = mybir.AluOpType

    nc = bacc.Bacc()
    xin = nc.dram_tensor("xin", [NCORES * NLOCP, H], bf16, kind="ExternalInput")
    gidx = nc.dram_tensor("gidx", [P, EPC // 16], i16, kind="ExternalInput")
    dstl = nc.dram_tensor("dstl", [P, NCHUNKS], f32, kind="ExternalInput")
    pgl = nc.dram_tensor("pgl", [P, NT], f32, kind="ExternalInput")
    w1t = nc.dram_tensor("w1t", [L, KT, P, H], bf16, kind="ExternalInput")
    w2t = nc.dram_tensor("w2t", [L, KT, P, H], bf16, kind="ExternalInput")
    b1t = nc.dram_tensor("b1t", [L, 1, H], bf16, kind="ExternalInput")
    b2t = nc.dram_tensor("b2t", [P, L * KT], f32, kind="ExternalInput")
    fc1wp = nc.dram_tensor("fc1wp", [KF, P, FO], f32, kind="ExternalInput")
    fc1bt = nc.dram_tensor("fc1bt", [P, MT], f32, kind="ExternalInput")
    fc2wp = nc.dram_tensor("fc2wp", [P, 2], f32, kind="ExternalInput")
    fc2bt = nc.dram_tensor("fc2bt", [1, 1], f32, kind="ExternalInput")
    statst = nc.dram_tensor("statst", [P, PGT], f32, kind="ExternalInput")
    adjt = nc.dram_tensor("adjt", [KADJ, P, PGT], f32, kind="ExternalInput")

    hb = nc.dram_tensor("hb", [NLOCP, H], bf16)
    gath = [None,
            nc.dram_tensor("gath1", [NCORES * NLOCP, H], bf16, addr_space="Shared"),
            nc.dram_tensor("gath2", [NCORES * NLOCP, H], bf16, addr_space="Shared")]

    out_g = nc.dram_tensor("out_g", [1, PGT], f32, kind="ExternalOutput")
    xl_g = nc.dram_tensor("xl_g", [PGT, FO], f32, kind="ExternalOutput")

    cc_sem = nc.alloc_semaphore("cc_sem")

    with tile.TileContext(nc) as tc:
        with (
            tc.tile_pool(name="const", bufs=1) as cp,
            tc.tile_pool(name="wpool", bufs=1) as wp,
            tc.tile_pool(name="msgs", bufs=3) as mp,
            tc.tile_pool(name="oh", bufs=4) as ohp,
            tc.tile_pool(name="work", bufs=2) as wk,
            tc.tile_pool(name="chunk", bufs=2) as ck,
            tc.tile_pool(name="psu", bufs=2, space="PSUM") as pp_u,
            tc.tile_pool(name="pst", bufs=1, space="PSUM") as pp_t,
            tc.tile_pool(name="psz", bufs=2, space="PSUM") as pp_z,
            tc.tile_pool(name="psh", bufs=1, space="PSUM") as pp_h,
            tc.tile_pool(name="psp", bufs=1, space="PSUM") as pp_p,
        ):
            # ---- constants / resident tensors
            gidx_sb = cp.tile([P, EPC // 16], i16)
            nc.sync.dma_start(gidx_sb[:], gidx[:])
            dstl_sb = cp.tile([P, NCHUNKS], f32)
            nc.sync.dma_start(dstl_sb[:], dstl[:])
            pgl_sb = cp.tile([P, NT], f32)
            nc.sync.dma_start(pgl_sb[:], pgl[:])
            iota_i = cp.tile([P, P], mybir.dt.int32)
            nc.gpsimd.iota(iota_i[:], pattern=[[1, P]], base=0, channel_multiplier=0)
            iota_bf = cp.tile([P, P], bf16)
            nc.vector.tensor_copy(iota_bf[:], iota_i[:])
            ident_bf = cp.tile([P, P], bf16)
            make_identity(nc, ident_bf[:])
            ident_f = cp.tile([P, P], f32)
            make_identity(nc, ident_f[:])
            ones_sb = cp.tile([1, 512], bf16)
            nc.gpsimd.memset(ones_sb[:], 1.0)

            w1_sb = wp.tile([P, L * KT * H], bf16)
            w2_sb = wp.tile([P, L * KT * H], bf16)
            for l_ in range(L):
                for kt in range(KT):
                    o = (l_ * KT + kt) * H
                    nc.sync.dma_start(w1_sb[:, o:o + H], w1t[l_, kt, :, :])
                    nc.sync.dma_start(w2_sb[:, o:o + H], w2t[l_, kt, :, :])
            b1_sb = wp.tile([1, L * H], bf16)
            for l_ in range(L):
                nc.sync.dma_start(b1_sb[0:1, l_ * H:(l_ + 1) * H], b1t[l_, :, :])
            b2_sb = wp.tile([P, L * KT], f32)
            nc.sync.dma_start(b2_sb[:], b2t[:])

            psum_pool = pp_p.tile([PGT, H], f32, space="PSUM")

            # ---- layers
            for l in range(L):
                src_d = xin if l == 0 else gath[l]
                a1l = meta["a1"][l]
                nbatch = (NCHUNKS + GB - 1) // GB
                msg_tiles = {}

                def get_msgs(b):
                    if b not in msg_tiles:
                        nb = min(GB, NCHUNKS - b * GB)
                        mt_ = mp.tile([P, GB, H], bf16, tag="msgs")
                        nc.gpsimd.dma_gather(
                            mt_[:, :nb, :], src_d[:],
                            gidx_sb[:, b * GB * CHUNK // 16:
                                    (b * GB + nb) * CHUNK // 16],
                            nb * CHUNK, nb * CHUNK, H, single_packet=False)
                        msg_tiles[b] = mt_
                        if len(msg_tiles) > 3:
                            del msg_tiles[min(msg_tiles)]
                    return msg_tiles[b]

                nchk = (NT + 3) // 4
                for blk in range(nchk):
                    t_lo = blk * 4
                    t_hi = min(t_lo + 4, NT)
                    ntile = t_hi - t_lo
                    fd = ntile * P
                    uTc = ck.tile([P, KT, 512], bf16, tag="uTc")
                    for t in range(t_lo, t_hi):
                        psum_u = pp_u.tile([P, H], f32, space="PSUM", tag="u")
                        for ci in range(nchunk_t[t]):
                            q = qt0[t] + ci
                            mt_ = get_msgs(q // GB)
                            oh = ohp.tile([P, P], bf16, tag="oh")
                            nc.vector.tensor_scalar(
                                out=oh[:], in0=iota_bf[:],
                                scalar1=dstl_sb[:, q:q + 1], scalar2=None,
                                op0=AL.is_equal)
                            nc.tensor.matmul(
                                psum_u[:], lhsT=oh[:], rhs=mt_[:, q % GB, :],
                                start=(ci == 0), stop=(ci == nchunk_t[t] - 1))
                        u_sb = wk.tile([P, H], bf16, tag="u_sb")
                        nc.scalar.activation(u_sb[:], psum_u[:],
                                             mybir.ActivationFunctionType.Copy)
                        psum_uT = pp_t.tile([P, H], bf16, space="PSUM", tag="uT")
                        for kt in range(KT):
                            nc.tensor.transpose(
                                psum_uT[:, kt * P:(kt + 1) * P],
                                u_sb[:, kt * P:(kt + 1) * P], ident_bf[:])
                        for kt in range(KT):
                            nc.scalar.activation(
                                uTc[:, kt, (t - t_lo) * P:(t - t_lo + 1) * P],
                                psum_uT[:, kt * P:(kt + 1) * P],
                                mybir.ActivationFunctionType.Copy)
                    # MLP on this node block (H-major, free dim = fd)
                    z1T = ck.tile([P, KT, 512], bf16, tag="z1T")
                    for mt_i in range(KT):
                        psum_z = pp_z.tile([P, 512], f32, space="PSUM", tag="z")
                        for kt in range(KT):
                            nc.tensor.matmul(
                                psum_z[:, :fd],
                                lhsT=w1_sb[:, (l * KT + kt) * H + mt_i * P:
                                           (l * KT + kt) * H + (mt_i + 1) * P],
                                rhs=uTc[:, kt, :fd], start=(kt == 0), stop=False)
                        nc.tensor.matmul(
                            psum_z[:, :fd],
                            lhsT=b1_sb[0:1, l * H + mt_i * P:l * H + (mt_i + 1) * P],
                            rhs=ones_sb[0:1, :fd], start=False, stop=True)
                        t2 = wk.tile([P, 512], bf16, tag="t2")
                        nc.vector.tensor_scalar(
                            out=t2[:, :fd], in0=psum_z[:, :fd], scalar1=a1l,
                            scalar2=None, op0=AL.mult)
                        nc.vector.tensor_tensor(
                            out=z1T[:, mt_i, :fd], in0=psum_z[:, :fd],
                            in1=t2[:, :fd], op=AL.max)
                    hT = ck.tile([P, KT, 512], bf16, tag="hT")
                    for mt_i in range(KT):
                        psum_z = pp_z.tile([P, 512], f32, space="PSUM", tag="z")
                        for kt in range(KT):
                            nc.tensor.matmul(
                                psum_z[:, :fd],
                                lhsT=w2_sb[:, (l * KT + kt) * H + mt_i * P:
                                           (l * KT + kt) * H + (mt_i + 1) * P],
                                rhs=z1T[:, kt, :fd],
                                start=(kt == 0), stop=(kt == KT - 1))
                        nc.scalar.activation(
                            hT[:, mt_i, :fd], psum_z[:, :fd],
                            mybir.ActivationFunctionType.Relu,
                            bias=b2_sb[:, l * KT + mt_i:l * KT + mt_i + 1])
                    # back-transpose to node-major; ship or pool
                    for ti in range(ntile):
                        t0 = t_lo + ti
                        psum_h = pp_h.tile([P, H], bf16, space="PSUM", tag="hn")
                        for kt in range(KT):
                            nc.tensor.transpose(
                                psum_h[:, kt * P:(kt + 1) * P],
                                hT[:, kt, ti * P:(ti + 1) * P], ident_bf[:])
                        h_sb = wk.tile([P, H], bf16, tag="h_sb")
                        nc.scalar.activation(h_sb[:], psum_h[:],
                                             mybir.ActivationFunctionType.Copy)
                        if l < L - 1:
                            nc.sync.dma_start(hb[t0 * P:(t0 + 1) * P, :], h_sb[:])
                        else:
                            poh = ohp.tile([P, PGT], bf16, tag="poh")
                            nc.vector.tensor_scalar(
                                out=poh[:], in0=iota_bf[:, :PGT],
                                scalar1=pgl_sb[:, t0:t0 + 1], scalar2=None,
                                op0=AL.is_equal)
                            nc.tensor.matmul(
                                psum_pool[:], lhsT=poh[:], rhs=h_sb[:],
                                start=(t0 == 0), stop=(t0 == NT - 1))
                if l < L - 1:
                    with tc.tile_critical():
                        nc.gpsimd.collective_compute(
                            "AllGather", AL.bypass,
                            replica_groups=[list(range(NCORES))],
                            ins=[hb[:]], outs=[gath[l + 1][:]],
                        ).then_inc(cc_sem, 1)
                        nc.gpsimd.wait_ge(cc_sem, l + 1)

            # ---- tail: pooled -> fc1 -> fc2
            pooled_sb = wk.tile([PGT, H], f32, tag="pooled")
            nc.scalar.activation(pooled_sb[:], psum_pool[:],
                                 mybir.ActivationFunctionType.Copy)
            pooledT = wk.tile([P, KT * PGT], f32, tag="pooledT")
            for kt in range(KT):
                psum_pt = pp_t.tile([P, PGT], f32, space="PSUM", tag="uT")
                nc.tensor.transpose(
                    psum_pt[:], pooled_sb[:, kt * P:(kt + 1) * P],
                    ident_f[:PGT, :PGT])
                nc.scalar.activation(pooledT[:, kt * PGT:(kt + 1) * PGT],
                                     psum_pt[:],
                                     mybir.ActivationFunctionType.Copy)
            statst_sb = wk.tile([P, PGT], f32, tag="statst")
            nc.sync.dma_start(statst_sb[:], statst[:])

            psum_x1 = []
            for _mt in range(MT):
                px = pp_z.tile([P, PGT], f32, space="PSUM", tag="z")
                psum_x1.append(px)
            for kf in range(KF):
                fw = mp.tile([P, FO], f32, tag="fcw")
                nc.sync.dma_start(fw[:], fc1wp[kf, :, :])
                if kf < KT:
                    rhs = pooledT[:, kf * PGT:(kf + 1) * PGT]
                elif kf == KT:
                    rhs = statst_sb[:]
                else:
                    at = mp.tile([P, PGT], f32, tag="adjt")
                    nc.sync.dma_start(at[:], adjt[kf - KT - 1, :, :])
                    rhs = at[:]
                for mt_i in range(MT):
                    nc.tensor.matmul(
                        psum_x1[mt_i][:], lhsT=fw[:, mt_i * P:(mt_i + 1) * P],
                        rhs=rhs, start=(kf == 0), stop=(kf == KF - 1))
            fc1b_sb = wk.tile([P, MT], f32, tag="fc1b")
            nc.sync.dma_start(fc1b_sb[:], fc1bt[:])
            xlT = wk.tile([P, MT * PGT], f32, tag="xlT")
            for mt_i in range(MT):
                nc.scalar.activation(
                    xlT[:, mt_i * PGT:(mt_i + 1) * PGT], psum_x1[mt_i][:],
                    mybir.ActivationFunctionType.Relu,
                    bias=fc1b_sb[:, mt_i:mt_i + 1])
            # fc2 + sigmoid
            fc2w_sb = wk.tile([P, 2], f32, tag="fc2w")
            nc.sync.dma_start(fc2w_sb[:], fc2wp[:])
            psum_o = pp_h.tile([1, PGT], f32, space="PSUM", tag="hn")
            for kt2 in range(MT):
                nc.tensor.matmul(
                    psum_o[:], lhsT=fc2w_sb[:, kt2:kt2 + 1],
                    rhs=xlT[:, kt2 * PGT:(kt2 + 1) * PGT],
                    start=(kt2 == 0), stop=(kt2 == MT - 1))
            fc2b_sb = wk.tile([1, 1], f32, tag="fc2b")
            nc.sync.dma_start(fc2b_sb[:], fc2bt[:])
            o_sb = wk.tile([1, PGT], f32, tag="o_sb")
            nc.scalar.activation(o_sb[:], psum_o[:],
                                 mybir.ActivationFunctionType.Sigmoid,
                                 bias=fc2b_sb[:])
            nc.sync.dma_start(out_g[:], o_sb[:])
            # x_l: transpose back to graph-major
            for mt_i in range(MT):
                psum_xl = pp_u.tile([PGT, P], f32, space="PSUM", tag="u")
                nc.tensor.transpose(
                    psum_xl[:], xlT[:, mt_i * PGT:(mt_i + 1) * PGT], ident_f[:])
                xl_sb = wk.tile([PGT, P], f32, tag="xl_sb")
                nc.scalar.activation(xl_sb[:], psum_xl[:],
                                     mybir.ActivationFunctionType.Copy)
                nc.sync.dma_start(xl_g[:, mt_i * P:(mt_i + 1) * P], xl_sb[:])

    nc.has_collectives = True
    nc.compile()
    return nc


# ----------------------------------------------------------------- runner
def _make_runner(nc, n_cores=NCORES):
    import jax
    from jax.sharding import Mesh, PartitionSpec
    from jax.experimental.shard_map import shard_map
    from concourse.bass2jax import (_bass_exec_p, install_neuronx_cc_hook,
                                    partition_id_tensor)
    install_neuronx_cc_hook()
    partition_name = nc.partition_id_tensor.name if nc.partition_id_tensor else None
    in_names, out_names, out_avals, zero_outs = [], [], [], []
    for alloc in nc.m.functions[0].allocations:
        if not isinstance(alloc, mybir.MemoryLocationSet):
            continue
        name = alloc.memorylocations[0].name
        if alloc.kind == "ExternalInput":
            if name != partition_name:
                in_names.append(name)
        elif alloc.kind == "ExternalOutput":
            shape = tuple(alloc.tensor_shape)
            dtype = mybir.dt.np(alloc.dtype)
            out_names.append(name)
            out_avals.append(jax.core.ShapedArray(shape, dtype))
            zero_outs.append(np.zeros(shape, dtype))
    n_params = len(in_names)
    n_outs = len(out_avals)
    all_in = list(in_names) + list(out_names)
    if partition_name is not None:
        all_in.append(partition_name)

    def _body(*args):
        operands = list(args)
        if partition_name is not None:
            operands.append(partition_id_tensor())
        return tuple(_bass_exec_p.bind(
            *operands, out_avals=tuple(out_avals), in_names=tuple(all_in),
            out_names=tuple(out_names), lowering_input_output_aliases=(),
            sim_require_finite=False, sim_require_nnan=False, nc=nc))

    devices = jax.devices()[:n_cores]
    mesh = Mesh(np.asarray(devices), ("core",))
    sharded = jax.jit(
        shard_map(_body, mesh=mesh,
                  in_specs=(PartitionSpec("core"),) * (n_params + n_outs),
                  out_specs=(PartitionSpec("core"),) * n_outs,
                  check_rep=False),
        donate_argnums=tuple(range(n_params, n_params + n_outs)),
        keep_unused=True)

    def run(in_maps):
        import jax as _jax
        per_core = [[np.asarray(m[k]) for k in in_names] for m in in_maps]
        concat_in = [np.concatenate([per_core[c][i] for c in range(n_cores)], axis=0)
                     for i in range(n_params)]
        zeros = [np.zeros((n_cores * z.shape[0], *z.shape[1:]), z.dtype)
                 for z in zero_outs]
        outs = sharded(*concat_in, *zeros)
        _jax.block_until_ready(outs)
        return [
            {k: np.asarray(outs[i]).reshape(n_cores, *out_avals[i].shape)[c]
             for i, k in enumerate(out_names)}
            for c in range(n_cores)
        ]
    run.in_names = in_names
    run.out_names = out_names
    run.zero_outs = zero_outs
    run.sharded = sharded
    return run


# ----------------------------------------------------------------- entry point
def kernel(**inputs):
    meta, in_maps = preprocess(inputs)
    nc = build_nc(meta)
    run = _make_runner(nc)
    res = run(in_maps)
    gper = meta["gper"]
    out = np.concatenate([res[c]["out_g"][0, :gper] for c in range(NCORES)])
    out = out.reshape(-1, 1).astype(np.float32)
    x_l = np.concatenate([res[c]["xl_g"][:gper, :] for c in range(NCORES)], axis=0)
    return out, x_l.astype(np.float32)


# revision 20
# speedup vs baseline: 1.9354x; 1.7392x over previous
"""Trainium2 Bass kernel for nn_Discriminator (GIN message passing + pool + FC).

Strategy (8 NeuronCores, SPMD):
- Nodes sharded by graph boundaries (graphs 50c..50(c+1) -> core c), so
  global_add_pool and the FC tail are fully core-local.
- Message passing: edges assigned to the core owning dst; edges sorted by dst;
  gather h[src] rows via SWDGE dma_gather (random 1KB rows, bf16); scatter-add
  via one-hot matmuls on the TensorEngine accumulating in PSUM. "+h" (GIN eps=0)
  is realized with self-edges, so u = h + A@h comes out of PSUM directly.
- MLP per layer runs H-major (activations transposed via PE transposes):
  z1 = PReLU(u@w1 + b1) [bias via K=1 matmul; PReLU = max(v, a*v) on DVE],
  h  = relu(z1@w2' + b2') [BatchNorm folded into w2/b2 on host; ACT Relu+bias].
- Between layers, h (bf16) is exchanged with an ncfw AllGather into a Shared
  DRAM buffer which doubles as the next layer's gather table.
- Tail: pooling via one-hot matmul (graph-local), FC1/FC2 with stats/adj
  pre-transposed on host; outputs per-core [50 graphs] slices, host concats.
"""
import sys
sys.path.insert(0, '/opt/trn_rl_repo')

import numpy as np
import ml_dtypes

import concourse.bacc as bacc
import concourse.mybir as mybir
import concourse.tile as tile
from concourse.masks import make_identity

P = 128
NCORES = 8
CHUNK = 128     # edges per scatter matmul
GB = 16         # chunks per dma_gather batch
L = 3
BN_EPS = 1e-5
PGT = 64        # padded graphs per core

BF = ml_dtypes.bfloat16
SKIP_CC = False


# ----------------------------------------------------------------- host prep
def preprocess(inputs):
    x = np.asarray(inputs["x"], np.float32)
    ei = np.asarray(inputs["edge_index"], np.int64)
    batch = np.asarray(inputs["batch"], np.int64)
    N, H = x.shape
    E = ei.shape[1]
    G = int(np.asarray(inputs["stats"]).shape[0])
    KT = H // P
    gper = G // NCORES
    assert G % NCORES == 0 and H % P == 0

    # --- graph-aligned node ranges
    n0 = np.searchsorted(batch, np.arange(NCORES) * gper, side="left")
    n1 = np.append(n0[1:], N)
    cnt = n1 - n0
    NT = int(np.ceil(cnt.max() / P))
    NLOCP = NT * P
    assert NCORES * NLOCP < 32768, "int16 gather index overflow"

    # padded-row mapping: global node -> padded row
    owner = np.searchsorted(n0, np.arange(N), side="right") - 1
    prow = owner * NLOCP + (np.arange(N) - n0[owner])

    x_pad = np.zeros((NCORES * NLOCP, H), np.float32)
    x_pad[prow] = x
    x_pad_bf = x_pad.astype(BF)

    # --- edges per core (dst-owner), with self-edges, sorted by dst
    src, dst = ei[0], ei[1]
    e_owner = owner[dst]
    # self edges
    all_src = np.concatenate([src, np.arange(N)])
    all_dst = np.concatenate([dst, np.arange(N)])
    all_owner = np.concatenate([e_owner, owner])

    nchunk_t = np.zeros(NT, np.int64)
    per_core = []
    for c in range(NCORES):
        m = all_owner == c
        s_c = prow[all_src[m]]
        d_c = prow[all_dst[m]] - c * NLOCP
        o = np.argsort(d_c, kind="stable")
        s_c, d_c = s_c[o], d_c[o]
        t_c = d_c // P
        counts = np.bincount(t_c, minlength=NT)
        nchunk_t = np.maximum(nchunk_t, (counts + CHUNK - 1) // CHUNK)
        per_core.append((s_c, d_c, counts))
    nchunk_t = np.maximum(nchunk_t, 1)
    NCHUNKS = int(nchunk_t.sum())
    EPC = NCHUNKS * CHUNK

    gidx = np.zeros((NCORES, EPC), np.int16)
    dstl = np.full((NCORES, EPC), -1.0, np.float32)
    qt0 = np.concatenate([[0], np.cumsum(nchunk_t)])[:-1]  # first chunk of tile t
    for c in range(NCORES):
        s_c, d_c, counts = per_core[c]
        off = np.concatenate([[0], np.cumsum(counts)])
        for t in range(NT):
            seg = slice(off[t], off[t + 1])
            k = counts[t]
            base = qt0[t] * CHUNK
            gidx[c, base:base + k] = s_c[seg]
            dstl[c, base:base + k] = (d_c[seg] - t * P).astype(np.float32)

    # half-slice split for collective/compute overlap
    nchk_b = (NT + 3) // 4
    TH = min(4 * max(nchk_b // 2, 1), NT)   # tiles in half 0
    ROWS0, ROWS1 = TH * P, NLOCP - TH * P

    # wrap indices: [128, EPC//16] int16 (16-partition wrap, replicated 8x)
    def wrap16(a):
        w = a.reshape(-1, 16).T  # [16, EPC/16]
        return np.tile(w, (8, 1)).copy()

    gidx_w = np.stack([wrap16(gidx[c]) for c in range(NCORES)])
    # layer>=1 gather layout: gath = [8 x ROWS0 | 8 x ROWS1] sections
    g_c = gidx // NLOCP
    g_loc = gidx - g_c * NLOCP
    gidx12 = np.where(g_loc < ROWS0, g_c * ROWS0 + g_loc,
                      NCORES * ROWS0 + g_c * ROWS1 + (g_loc - ROWS0)).astype(np.int16)
    gidx12_w = np.stack([wrap16(gidx12[c]) for c in range(NCORES)])
    # dstl per chunk column: [128, NCHUNKS]
    dstl_t = dstl.reshape(NCORES, NCHUNKS, CHUNK).transpose(0, 2, 1).astype(np.float32).copy()

    # pool graph-locals per node row: [128, NT]
    pgl = np.full((NCORES, NLOCP), -1.0, np.float32)
    for c in range(NCORES):
        g_loc = batch[n0[c]:n1[c]] - gper * c
        assert g_loc.min() >= 0 and g_loc.max() < PGT
        pgl[c, :cnt[c]] = g_loc
    pgl_t = pgl.reshape(NCORES, NT, P).transpose(0, 2, 1).astype(np.float32).copy()

    # --- weights
    w1 = np.asarray(inputs["w1"], np.float64)
    b1 = np.asarray(inputs["b1"], np.float64)
    a1 = np.asarray(inputs["a1"], np.float64)
    gam = np.asarray(inputs["bn_gamma"], np.float64)
    bet = np.asarray(inputs["bn_beta"], np.float64)
    mu = np.asarray(inputs["bn_mean"], np.float64)
    var = np.asarray(inputs["bn_var"], np.float64)
    w2 = np.asarray(inputs["w2"], np.float64)
    b2 = np.asarray(inputs["b2"], np.float64)
    a2 = np.asarray(inputs["a2"], np.float64)
    assert np.all(a2 >= 0), "relu(prelu) fusion needs a2 >= 0"

    s = gam / np.sqrt(var + BN_EPS)          # [L, H]
    tt = bet - mu * s
    w2f = s[:, :, None] * w2                 # fold BN into w2
    b2f = np.einsum("lh,lho->lo", tt, w2) + b2

    w1t = w1.reshape(L, KT, P, H).astype(BF)
    w2t = w2f.reshape(L, KT, P, H).astype(BF)
    b1t = b1.reshape(L, 1, H).astype(BF)
    b2t = np.ascontiguousarray(
        b2f.reshape(L, KT, P).transpose(2, 0, 1).reshape(P, L * KT)
    ).astype(np.float32)

    # --- fc tail
    fc1_w = np.asarray(inputs["fc1_w"], np.float32)   # [H + 8 + NMAX^2, 256]
    fc1_b = np.asarray(inputs["fc1_b"], np.float32)
    fc2_w = np.asarray(inputs["fc2_w"], np.float32)   # [256, 1]
    fc2_b = float(np.asarray(inputs["fc2_b"]).reshape(-1)[0])
    stats = np.asarray(inputs["stats"], np.float32)
    adj = np.asarray(inputs["adj"], np.float32)
    NC2 = stats.shape[1]
    AD = adj.shape[1] * adj.shape[2]
    FO = fc1_w.shape[1]
    ADP = ((AD + P - 1) // P) * P
    KADJ = ADP // P
    KF = KT + 1 + KADJ
    fc1wp = np.zeros((KF * P, FO), np.float32)
    fc1wp[:H] = fc1_w[:H]
    fc1wp[H:H + NC2] = fc1_w[H:H + NC2]
    fc1wp[H + P:H + P + AD] = fc1_w[H + NC2:]
    fc1wp = fc1wp.reshape(KF, P, FO)
    MT = FO // P
    fc1bt = np.ascontiguousarray(fc1_b.reshape(MT, P).T).astype(np.float32)
    fc2wp = np.ascontiguousarray(fc2_w.reshape(2, P).T).astype(np.float32)

    statst = np.zeros((NCORES, P, PGT), np.float32)
    adjt = np.zeros((NCORES, KADJ, P, PGT), np.float32)
    for c in range(NCORES):
        statst[c, :NC2, :gper] = stats[gper * c:gper * (c + 1)].T
        a = adj[gper * c:gper * (c + 1)].reshape(gper, AD).T  # [AD, gper]
        adjt[c, :, :, :gper] = np.pad(a, ((0, ADP - AD), (0, 0))).reshape(KADJ, P, gper)

    # ---- pack per-core inputs into few tensors (axon arg overhead ~0.2ms/tensor)
    EPC16 = EPC // 16
    # f32 pack columns: dstl | pgl | b2 | fc1b | fc2w | fc2b | stats | adj | fc1w
    o_dstl = 0
    o_pgl = o_dstl + NCHUNKS
    o_b2 = o_pgl + NT
    o_fc1b = o_b2 + L * KT
    o_fc2w = o_fc1b + MT
    o_fc2b = o_fc2w + 2
    o_stats = o_fc2b + 1
    o_adj = o_stats + PGT
    o_fc1w = o_adj + KADJ * PGT
    FCOLS = o_fc1w + KF * FO
    pkf = np.zeros((NCORES, P, FCOLS), np.float32)
    for c in range(NCORES):
        pkf[c, :, o_dstl:o_dstl + NCHUNKS] = dstl_t[c]
        pkf[c, :, o_pgl:o_pgl + NT] = pgl_t[c]
        pkf[c, :, o_b2:o_b2 + L * KT] = b2t
        pkf[c, :, o_fc1b:o_fc1b + MT] = fc1bt
        pkf[c, :, o_fc2w:o_fc2w + 2] = fc2wp
        pkf[c, 0, o_fc2b] = fc2_b
        pkf[c, :, o_stats:o_stats + PGT] = statst[c]
        pkf[c, :, o_adj:o_adj + KADJ * PGT] = adjt[c].transpose(1, 0, 2).reshape(P, KADJ * PGT)
        pkf[c, :, o_fc1w:] = fc1wp.transpose(1, 0, 2).reshape(P, KF * FO)
    # bf16 pack: w1 | w2 | b1 (b1 lives in row 0)
    o_w1, o_w2, o_b1 = 0, L * KT * H, 2 * L * KT * H
    BCOLS = o_b1 + L * H
    pkbf = np.zeros((P, BCOLS), BF)
    pkbf[:, o_w1:o_w1 + L * KT * H] = w1t.transpose(2, 0, 1, 3).reshape(P, L * KT * H)
    pkbf[:, o_w2:o_w2 + L * KT * H] = w2t.transpose(2, 0, 1, 3).reshape(P, L * KT * H)
    pkbf[0, o_b1:] = b1t.reshape(L * H)
    # int16 pack: gidx | gidx12
    pk16 = np.concatenate([gidx_w, gidx12_w], axis=2)  # [NCORES, P, 2*EPC16]

    meta = dict(
        N=N, H=H, KT=KT, G=G, gper=gper, NT=NT, NLOCP=NLOCP,
        NCHUNKS=NCHUNKS, EPC=EPC, nchunk_t=nchunk_t.tolist(),
        qt0=qt0.tolist(), a1=[float(v) for v in a1], fc2_b=fc2_b,
        TH=TH, ROWS0=ROWS0, ROWS1=ROWS1,
        KF=KF, KADJ=KADJ, MT=MT, FO=FO, cnt=cnt.tolist(),
        EPC16=EPC16, FCOLS=FCOLS, BCOLS=BCOLS,
        o=dict(dstl=o_dstl, pgl=o_pgl, b2=o_b2, fc1b=o_fc1b, fc2w=o_fc2w,
               fc2b=o_fc2b, stats=o_stats, adj=o_adj, fc1w=o_fc1w,
               w1=o_w1, w2=o_w2, b1=o_b1),
    )
    in_maps = [dict(xin=x_pad_bf, pkbf=pkbf, pkf=pkf[c], pk16=pk16[c])
               for c in range(NCORES)]
    return meta, in_maps


# ----------------------------------------------------------------- device build
def build_nc(meta):
    H, KT, NT, NLOCP = meta["H"], meta["KT"], meta["NT"], meta["NLOCP"]
    NCHUNKS, EPC = meta["NCHUNKS"], meta["EPC"]
    nchunk_t, qt0 = meta["nchunk_t"], meta["qt0"]
    KF, KADJ, MT, FO = meta["KF"], meta["KADJ"], meta["MT"], meta["FO"]
    TH, ROWS0, ROWS1 = meta["TH"], meta["ROWS0"], meta["ROWS1"]
    f32, bf16, i16 = mybir.dt.float32, mybir.dt.bfloat16, mybir.dt.int16
    AL = mybir.AluOpType

    EPC16 = meta["EPC16"]
    FCOLS, BCOLS, O = meta["FCOLS"], meta["BCOLS"], meta["o"]
    nc = bacc.Bacc()
    xin = nc.dram_tensor("xin", [NCORES * NLOCP, H], bf16, kind="ExternalInput")
    pk16 = nc.dram_tensor("pk16", [P, 2 * EPC16], i16, kind="ExternalInput")
    pkbf_d = nc.dram_tensor("pkbf", [P, BCOLS], bf16, kind="ExternalInput")
    pkf_d = nc.dram_tensor("pkf", [P, FCOLS], f32, kind="ExternalInput")

    hb = nc.dram_tensor("hb", [NLOCP, H], bf16)
    gath = [None,
            nc.dram_tensor("gath1", [NCORES * NLOCP, H], bf16, addr_space="Shared"),
            nc.dram_tensor("gath2", [NCORES * NLOCP, H], bf16, addr_space="Shared")]

    out_g = nc.dram_tensor("out_g", [1, PGT], f32, kind="ExternalOutput")
    xl_g = nc.dram_tensor("xl_g", [PGT, FO], f32, kind="ExternalOutput")

    cc_sem = nc.alloc_semaphore("cc_sem")

    with tile.TileContext(nc) as tc:
        with (
            tc.tile_pool(name="const", bufs=1) as cp,
            tc.tile_pool(name="wpool", bufs=1) as wp,
            tc.tile_pool(name="msgs", bufs=4) as mp,
            tc.tile_pool(name="oh", bufs=8) as ohp,
            tc.tile_pool(name="work", bufs=3) as wk,
            tc.tile_pool(name="chunk", bufs=3) as ck,
            tc.tile_pool(name="psu", bufs=2, space="PSUM") as pp_u,
            tc.tile_pool(name="pst", bufs=3, space="PSUM") as pp_t,
            tc.tile_pool(name="psz", bufs=2, space="PSUM") as pp_z,
            tc.tile_pool(name="psp", bufs=1, space="PSUM") as pp_p,
        ):
            # ---- constants / resident tensors
            pk16_sb = cp.tile([P, 2 * EPC16], i16)
            nc.sync.dma_start(pk16_sb[:], pk16[:])
            pkf_sb = cp.tile([P, FCOLS], f32)
            nc.sync.dma_start(pkf_sb[:], pkf_d[:])
            pkbf_sb = cp.tile([P, BCOLS], bf16)
            nc.sync.dma_start(pkbf_sb[:], pkbf_d[:])
            gidx_sb = pk16_sb[:, 0:EPC16]
            gidx12_sb = pk16_sb[:, EPC16:2 * EPC16]
            dstl_sb = pkf_sb[:, O["dstl"]:O["dstl"] + NCHUNKS]
            pgl_sb = pkf_sb[:, O["pgl"]:O["pgl"] + NT]
            iota_i = cp.tile([P, P], mybir.dt.int32)
            nc.gpsimd.iota(iota_i[:], pattern=[[1, P]], base=0, channel_multiplier=0)
            iota_bf = cp.tile([P, P], bf16)
            nc.vector.tensor_copy(iota_bf[:], iota_i[:])
            ident_bf = cp.tile([P, P], bf16)
            make_identity(nc, ident_bf[:])
            ident_f = cp.tile([P, P], f32)
            make_identity(nc, ident_f[:])
            ones_sb = cp.tile([1, 512], bf16)
            nc.gpsimd.memset(ones_sb[:], 1.0)

            w1_sb = pkbf_sb[:, O["w1"]:O["w1"] + L * KT * H]
            w2_sb = pkbf_sb[:, O["w2"]:O["w2"] + L * KT * H]
            b1_sb = pkbf_sb[0:1, O["b1"]:O["b1"] + L * H]
            b2_sb = pkf_sb[:, O["b2"]:O["b2"] + L * KT]

            psum_pool = pp_p.tile([PGT, H], f32, space="PSUM")

            # ---- layers
            for l in range(L):
                src_d = xin if l == 0 else gath[l]
                idx_sb = gidx_sb if l == 0 else gidx12_sb
                a1l = meta["a1"][l]
                nbatch = (NCHUNKS + GB - 1) // GB
                msg_tiles = {}

                def get_msgs(b):
                    if b not in msg_tiles:
                        nb = min(GB, NCHUNKS - b * GB)
                        mt_ = mp.tile([P, GB, H], bf16, tag="msgs")
                        nc.gpsimd.dma_gather(
                            mt_[:, :nb, :], src_d[:],
                            idx_sb[:, b * GB * CHUNK // 16:
                                   (b * GB + nb) * CHUNK // 16],
                            nb * CHUNK, nb * CHUNK, H, single_packet=False)
                        msg_tiles[b] = mt_
                        if len(msg_tiles) > 4:
                            del msg_tiles[min(msg_tiles)]
                    return msg_tiles[b]

                nchk = (NT + 3) // 4
                for blk in range(nchk):
                    t_lo = blk * 4
                    t_hi = min(t_lo + 4, NT)
                    ntile = t_hi - t_lo
                    fd = ntile * P
                    uTc = ck.tile([P, KT, 512], bf16, tag="uTc")
                    for t in range(t_lo, t_hi):
                        psum_u = pp_u.tile([P, H], f32, space="PSUM", tag="u")
                        for ci in range(nchunk_t[t]):
                            q = qt0[t] + ci
                            mt_ = get_msgs(q // GB)
                            oh = ohp.tile([P, P], bf16, tag="oh")
                            nc.vector.tensor_scalar(
                                out=oh[:], in0=iota_bf[:],
                                scalar1=dstl_sb[:, q:q + 1], scalar2=None,
                                op0=AL.is_equal)
                            nc.tensor.matmul(
                                psum_u[:], lhsT=oh[:], rhs=mt_[:, q % GB, :],
                                start=(ci == 0), stop=(ci == nchunk_t[t] - 1))
                        u_sb = wk.tile([P, H], bf16, tag="u_sb")
                        nc.scalar.activation(u_sb[:], psum_u[:],
                                             mybir.ActivationFunctionType.Copy)
                        if SKIP_MLP:
                            if l < L - 1:
                                nc.sync.dma_start(hb[t * P:(t + 1) * P, :], u_sb[:])
                            continue
                        psum_uT = pp_t.tile([P, H], bf16, space="PSUM", tag="uT")
                        for kt in range(KT):
                            nc.tensor.transpose(
                                psum_uT[:, kt * P:(kt + 1) * P],
                                u_sb[:, kt * P:(kt + 1) * P], ident_bf[:])
                        for kt in range(KT):
                            nc.scalar.activation(
                                uTc[:, kt, (t - t_lo) * P:(t - t_lo + 1) * P],
                                psum_uT[:, kt * P:(kt + 1) * P],
                                mybir.ActivationFunctionType.Copy)
                    if SKIP_MLP:
                        continue
                    # MLP on this node block (H-major, free dim = fd)
                    z1T = ck.tile([P, KT, 512], bf16, tag="z1T")
                    for mt_i in range(KT):
                        psum_z = pp_z.tile([P, 512], f32, space="PSUM", tag="z")
                        for kt in range(KT):
                            nc.tensor.matmul(
                                psum_z[:, :fd],
                                lhsT=w1_sb[:, (l * KT + kt) * H + mt_i * P:
                                           (l * KT + kt) * H + (mt_i + 1) * P],
                                rhs=uTc[:, kt, :fd], start=(kt == 0), stop=False)
                        nc.tensor.matmul(
                            psum_z[:, :fd],
                            lhsT=b1_sb[0:1, l * H + mt_i * P:l * H + (mt_i + 1) * P],
                            rhs=ones_sb[0:1, :fd], start=False, stop=True)
                        t2 = wk.tile([P, 512], bf16, tag="t2")
                        nc.vector.tensor_scalar(
                            out=t2[:, :fd], in0=psum_z[:, :fd], scalar1=a1l,
                            scalar2=None, op0=AL.mult)
                        nc.vector.tensor_tensor(
                            out=z1T[:, mt_i, :fd], in0=psum_z[:, :fd],
                            in1=t2[:, :fd], op=AL.max)
                    hT = ck.tile([P, KT, 512], bf16, tag="hT")
                    for mt_i in range(KT):
                        psum_z = pp_z.tile([P, 512], f32, space="PSUM", tag="z")
                        for kt in range(KT):
                            nc.tensor.matmul(
                                psum_z[:, :fd],
                                lhsT=w2_sb[:, (l * KT + kt) * H + mt_i * P:
                                           (l * KT + kt) * H + (mt_i + 1) * P],
                                rhs=z1T[:, kt, :fd],
                                start=(kt == 0), stop=(kt == KT - 1))
                        nc.scalar.activation(
                            hT[:, mt_i, :fd], psum_z[:, :fd],
                            mybir.ActivationFunctionType.Relu,
                            bias=b2_sb[:, l * KT + mt_i:l * KT + mt_i + 1])
                    # back-transpose to node-major; ship or pool
                    for ti in range(ntile):
                        t0 = t_lo + ti
                        psum_h = pp_t.tile([P, H], bf16, space="PSUM", tag="uT")
                        for kt in range(KT):
                            nc.tensor.transpose(
                                psum_h[:, kt * P:(kt + 1) * P],
                                hT[:, kt, ti * P:(ti + 1) * P], ident_bf[:])
                        h_sb = wk.tile([P, H], bf16, tag="h_sb")
                        nc.scalar.activation(h_sb[:], psum_h[:],
                                             mybir.ActivationFunctionType.Copy)
                        if l < L - 1:
                            nc.sync.dma_start(hb[t0 * P:(t0 + 1) * P, :], h_sb[:])
                        else:
                            poh = ohp.tile([P, PGT], bf16, tag="poh")
                            nc.vector.tensor_scalar(
                                out=poh[:], in0=iota_bf[:, :PGT],
                                scalar1=pgl_sb[:, t0:t0 + 1], scalar2=None,
                                op0=AL.is_equal)
                            nc.tensor.matmul(
                                psum_pool[:], lhsT=poh[:], rhs=h_sb[:],
                                start=(t0 == 0), stop=(t0 == NT - 1))
                    if (l < L - 1 and not SKIP_CC and not SKIP_MLP
                            and ROWS1 > 0 and t_hi == TH):
                        with tc.tile_critical():
                            nc.gpsimd.collective_compute(
                                "AllGather", AL.bypass,
                                replica_groups=[list(range(NCORES))],
                                ins=[hb[0:ROWS0, :]],
                                outs=[gath[l + 1][0:NCORES * ROWS0, :]],
                            ).then_inc(cc_sem, 1)
                            cc_count[0] += 1
                # (AG halves handled below)
                if l < L - 1 and not SKIP_CC:
                    split = (not SKIP_MLP) and ROWS1 > 0
                    with tc.tile_critical():
                        nc.gpsimd.collective_compute(
                            "AllGather", AL.bypass,
                            replica_groups=[list(range(NCORES))],
                            ins=[hb[ROWS0:, :] if split else hb[:]],
                            outs=[gath[l + 1][NCORES * ROWS0:, :] if split
                                  else gath[l + 1][:]],
                        ).then_inc(cc_sem, 1)
                        cc_count[0] += 1
                        nc.gpsimd.wait_ge(cc_sem, cc_count[0])

            # ---- tail: pooled -> fc1 -> fc2
            pooled_sb = wk.tile([PGT, H], f32, tag="pooled")
            nc.scalar.activation(pooled_sb[:], psum_pool[:],
                                 mybir.ActivationFunctionType.Copy)
            pooledT = wk.tile([P, KT * PGT], f32, tag="pooledT")
            for kt in range(KT):
                psum_pt = pp_t.tile([P, PGT], f32, space="PSUM", tag="uT")
                nc.tensor.transpose(
                    psum_pt[:], pooled_sb[:, kt * P:(kt + 1) * P],
                    ident_f[:PGT, :PGT])
                nc.scalar.activation(pooledT[:, kt * PGT:(kt + 1) * PGT],
                                     psum_pt[:],
                                     mybir.ActivationFunctionType.Copy)
            psum_x1 = []
            for _mt in range(MT):
                px = pp_z.tile([P, PGT], f32, space="PSUM", tag="z")
                psum_x1.append(px)
            for kf in range(KF):
                fw = pkf_sb[:, O["fc1w"] + kf * FO:O["fc1w"] + (kf + 1) * FO]
                if kf < KT:
                    rhs = pooledT[:, kf * PGT:(kf + 1) * PGT]
                elif kf == KT:
                    rhs = pkf_sb[:, O["stats"]:O["stats"] + PGT]
                else:
                    ka = kf - KT - 1
                    rhs = pkf_sb[:, O["adj"] + ka * PGT:O["adj"] + (ka + 1) * PGT]
                for mt_i in range(MT):
                    nc.tensor.matmul(
                        psum_x1[mt_i][:], lhsT=fw[:, mt_i * P:(mt_i + 1) * P],
                        rhs=rhs, start=(kf == 0), stop=(kf == KF - 1))
            fc1b_sb = pkf_sb[:, O["fc1b"]:O["fc1b"] + MT]
            xlT = wk.tile([P, MT * PGT], f32, tag="xlT")
            for mt_i in range(MT):
                nc.scalar.activation(
                    xlT[:, mt_i * PGT:(mt_i + 1) * PGT], psum_x1[mt_i][:],
                    mybir.ActivationFunctionType.Relu,
                    bias=fc1b_sb[:, mt_i:mt_i + 1])
            # fc2 + sigmoid
            fc2w_sb = pkf_sb[:, O["fc2w"]:O["fc2w"] + 2]
            psum_o = pp_t.tile([1, PGT], f32, space="PSUM", tag="uT")
            for kt2 in range(MT):
                nc.tensor.matmul(
                    psum_o[:], lhsT=fc2w_sb[:, kt2:kt2 + 1],
                    rhs=xlT[:, kt2 * PGT:(kt2 + 1) * PGT],
                    start=(kt2 == 0), stop=(kt2 == MT - 1))
            fc2b_sb = pkf_sb[0:1, O["fc2b"]:O["fc2b"] + 1]
            o_sb = wk.tile([1, PGT], f32, tag="o_sb")
            nc.scalar.activation(o_sb[:], psum_o[:],
                                 mybir.ActivationFunctionType.Sigmoid,
                                 bias=fc2b_sb[:])
            nc.sync.dma_start(out_g[:], o_sb[:])
            # x_l: transpose back to graph-major
            for mt_i in range(MT):
                psum_xl = pp_u.tile([PGT, P], f32, space="PSUM", tag="u")
                nc.tensor.transpose(
                    psum_xl[:], xlT[:, mt_i * PGT:(mt_i + 1) * PGT], ident_f[:])
                xl_sb = wk.tile([PGT, P], f32, tag="xl_sb")
                nc.scalar.activation(xl_sb[:], psum_xl[:],
                                     mybir.ActivationFunctionType.Copy)
                nc.sync.dma_start(xl_g[:, mt_i * P:(mt_i + 1) * P], xl_sb[:])

    nc.has_collectives = True
    nc.compile()
    return nc


# ----------------------------------------------------------------- runner
def _make_runner(nc, n_cores=NCORES):
    import jax
    from jax.sharding import Mesh, PartitionSpec
    from jax.experimental.shard_map import shard_map
    from concourse.bass2jax import (_bass_exec_p, install_neuronx_cc_hook,
                                    partition_id_tensor)
    install_neuronx_cc_hook()
    partition_name = nc.partition_id_tensor.name if nc.partition_id_tensor else None
    in_names, out_names, out_avals, zero_outs = [], [], [], []
    for alloc in nc.m.functions[0].allocations:
        if not isinstance(alloc, mybir.MemoryLocationSet):
            continue
        name = alloc.memorylocations[0].name
        if alloc.kind == "ExternalInput":
            if name != partition_name:
                in_names.append(name)
        elif alloc.kind == "ExternalOutput":
            shape = tuple(alloc.tensor_shape)
            dtype = mybir.dt.np(alloc.dtype)
            out_names.append(name)
            out_avals.append(jax.core.ShapedArray(shape, dtype))
            zero_outs.append(np.zeros(shape, dtype))
    n_params = len(in_names)
    n_outs = len(out_avals)
    all_in = list(in_names) + list(out_names)
    if partition_name is not None:
        all_in.append(partition_name)

    def _body(*args):
        operands = list(args)
        if partition_name is not None:
            operands.append(partition_id_tensor())
        return tuple(_bass_exec_p.bind(
            *operands, out_avals=tuple(out_avals), in_names=tuple(all_in),
            out_names=tuple(out_names), lowering_input_output_aliases=(),
            sim_require_finite=False, sim_require_nnan=False, nc=nc))

    devices = jax.devices()[:n_cores]
    mesh = Mesh(np.asarray(devices), ("core",))
    sharded = jax.jit(
        shard_map(_body, mesh=mesh,
                  in_specs=(PartitionSpec("core"),) * (n_params + n_outs),
                  out_specs=(PartitionSpec("core"),) * n_outs,
                  check_rep=False),
        donate_argnums=tuple(range(n_params, n_params + n_outs)),
        keep_unused=True)

    def run(in_maps):
        import jax as _jax
        per_core = [[np.asarray(m[k]) for k in in_names] for m in in_maps]
        concat_in = [np.concatenate([per_core[c][i] for c in range(n_cores)], axis=0)
                     for i in range(n_params)]
        zeros = [np.zeros((n_cores * z.shape[0], *z.shape[1:]), z.dtype)
                 for z in zero_outs]
        outs = sharded(*concat_in, *zeros)
        _jax.block_until_ready(outs)
        return [
            {k: np.asarray(outs[i]).reshape(n_cores, *out_avals[i].shape)[c]
             for i, k in enumerate(out_names)}
            for c in range(n_cores)
        ]
    run.in_names = in_names
    run.out_names = out_names
    run.zero_outs = zero_outs
    run.sharded = sharded
    return run


# ----------------------------------------------------------------- entry point
def kernel(**inputs):
    meta, in_maps = preprocess(inputs)
    nc = build_nc(meta)
    run = _make_runner(nc)
    res = run(in_maps)
    gper = meta["gper"]
    out = np.concatenate([res[c]["out_g"][0, :gper] for c in range(NCORES)])
    out = out.reshape(-1, 1).astype(np.float32)
    x_l = np.concatenate([res[c]["xl_g"][:gper, :] for c in range(NCORES)], axis=0)
    return out, x_l.astype(np.float32)
